# revision 33
# baseline (speedup 1.0000x reference)
"""Bass/Trainium2 kernel for nn_EuclideanGraphEncoder (GCN message passing).

Strategy: data-parallel over the batch (4 graphs per core, 8 cores),
weights replicated, no collectives. The adjacency matrix is transposed
and downcast to fp16 on the host during sharding so the aggregation
matmul (adj @ msg) can contract over SBUF partitions directly; per-layer
biases enter the aggregation PSUM as rank-1 matmuls against host-computed
exact row-sums of adj.

Device-side layout: h is kept transposed [hid=128 partitions, n=1024]
in SBUF fp16. Per layer:
  msg[n,k]  = h @ Wl       (8x K=128/M=128/N=128 matmuls, PSUM->SBUF fp16)
  aggT[k,n] = msg.T @ adjT (2 n-tiles x 8 accumulating K=128/N=512 matmuls
                            + 1 rank-1 bias matmul each)
  hT        = relu(aggT)   (ACT, PSUM->SBUF fp16 cast)
Projection returns to natural [n, 64] layout with a rank-1 bias matmul,
the node mask applied as a per-partition ACT scale, output fp32.
"""

import sys
from contextlib import ExitStack

import numpy as np

try:
    import concourse.bass as bass
except ImportError:  # fall back to the repo checkout
    sys.path.insert(0, "/opt/trn_rl_repo")
    import concourse.bass as bass

import concourse.tile as tile
from concourse import bacc, mybir
from concourse.bass_utils import run_bass_kernel_spmd

B, N, IN_DIM, HID, OUT = 32, 1024, 64, 128, 64
NUM_LAYERS = 3
N_CORES = 8
BPC = B // N_CORES  # graphs per core
NT = N // 512  # aggregation free-dim tiles
NC8 = N // 128  # node chunks of 128

FP16 = mybir.dt.float16
FP32 = mybir.dt.float32
RELU = mybir.ActivationFunctionType.Relu
COPY = mybir.ActivationFunctionType.Copy

# Per-layer power-of-2 scales: SBUF h/msg tiles hold h_true / S[i] so fp16
# never overflows (true agg magnitudes reach ~5e6). Scale hops are exact
# (powers of two) and ride existing instructions: the relu ACT scale, the
# host-prescaled bias operands, and the final mask scale (fp32).
S = [1.0, 64.0, 16384.0, 4194304.0]
ONES_VAL = 2.0 ** -11  # proj bias rank-1: ones * (b_proj * 2^11 / S[3])


def _kernel_body(ctx, tc, out, adjT, xT, maskT, w_embed, wl, blT, w_proj, b_proj):
    nc = tc.nc

    # All-resident pools (bufs = total tiles): load slots are write-once, so
    # load DMAs carry only their queue-ordering wait. adj is loaded as
    # per-chunk tiles so each aggregation matmul releases as soon as its own
    # 256KB slab lands instead of waiting for a whole graph (or all graphs).
    consts = ctx.enter_context(tc.tile_pool(name="consts", bufs=1))
    adj_pool = ctx.enter_context(tc.tile_pool(name="adj", bufs=BPC * NC8))
    xt_pool = ctx.enter_context(tc.tile_pool(name="xt", bufs=BPC))
    mask_pool = ctx.enter_context(tc.tile_pool(name="mask", bufs=BPC))
    h_pool = ctx.enter_context(tc.tile_pool(name="h", bufs=4))
    msg_pool = ctx.enter_context(tc.tile_pool(name="msg", bufs=3))
    o_pool = ctx.enter_context(tc.tile_pool(name="o", bufs=2))
    psA = ctx.enter_context(tc.tile_pool(name="psA", bufs=3, space="PSUM"))
    psM = ctx.enter_context(tc.tile_pool(name="psM", bufs=3, space="PSUM"))
    psO = ctx.enter_context(tc.tile_pool(name="psO", bufs=2, space="PSUM"))

    # Load-order = HW queue FIFO order. The compute-critical small tensors
    # (x of graph 0, then the weights, then the remaining x) go first on the
    # HWDGE queues, ahead of the 8MB adj flood; masks ride SWDGE (gpsimd),
    # they aren't needed until projection time.
    xts, masks = [], []
    for bb in range(BPC):
        xts.append(xt_pool.tile([IN_DIM, N], FP16, tag="xt", name=f"xt{bb}"))
    nc.sync.dma_start(xts[0][:], xT[0])
    we_t = consts.tile([IN_DIM, HID], FP16, tag="we")
    nc.sync.dma_start(we_t[:], w_embed[:, :])
    wl_t = []
    bl_t = []
    for i in range(NUM_LAYERS):
        w = consts.tile([HID, HID], FP16, tag=f"wl{i}")
        nc.sync.dma_start(w[:], wl[i])
        wl_t.append(w)
        # bias broadcast across partitions: A@(msg+b) == A@msg + rowsum(x)b,
        # so adding b to msg on the PSUM->SBUF copy replaces the rank-1
        # rowsum matmuls exactly. Layer-0 bias loads ahead of the adj
        # flood; later layers' biases are queued behind graph 0's adj.
        b = consts.tile([128, HID], FP32, tag=f"bl{i}")
        bl_t.append(b)
    nc.sync.dma_start(bl_t[0][:], blT[0].to_broadcast([128, HID]))
    ones_t = consts.tile([1, HID], FP16, tag="ones")
    nc.vector.memset(ones_t[:], ONES_VAL)

    # PE clock pre-warm: ~3.4us of dependency-free matmuls during the DMA
    # ramp flips the HAM throttle (1.2 -> 2.4 GHz) before real work arrives.
    warm_t = consts.tile([1, 512], FP16, tag="warm")
    nc.vector.memset(warm_t[:], 0.0)
    for _ in range(8):
        psw = psA.tile([HID, 512], FP32, tag="psA", name="psw")
        nc.tensor.matmul(psw[:], ones_t[:], warm_t[:], start=True, stop=True)

    def load_adj(bb):
        chunks = [
            adj_pool.tile([128, N], FP16, tag="adj", name=f"adj{bb}_{c}")
            for c in range(NC8)
        ]
        # two half-loads per chunk, all t=0 halves queued before t=1: the
        # first aggregation n-tile releases at half the graph transfer.
        for t in range(NT):
            for c in range(NC8):
                nc.sync.dma_start(
                    chunks[c][:, t * 512:(t + 1) * 512],
                    adjT[bb, c * 128:(c + 1) * 128, t * 512:(t + 1) * 512])
        return chunks

    adjs = [load_adj(0)]
    # everything not needed in graph 0's first ~15us queues after its adj
    for i in range(1, NUM_LAYERS):
        nc.sync.dma_start(bl_t[i][:], blT[i].to_broadcast([128, HID]))
    wp_t = consts.tile([HID, OUT], FP16, tag="wp")
    nc.sync.dma_start(wp_t[:], w_proj[:, :])
    bp_t = consts.tile([1, OUT], FP16, tag="bp")
    nc.sync.dma_start(bp_t[:], b_proj[:, :])
    for bb in range(1, BPC):
        nc.sync.dma_start(xts[bb][:], xT[bb])
    for bb in range(BPC):
        mask_t = mask_pool.tile([128, NC8], FP32, tag="mask", name=f"mask{bb}")
        nc.gpsimd.dma_start(mask_t[:], maskT[bb])
        masks.append(mask_t)
    for bb in range(1, BPC):
        adjs.append(load_adj(bb))

    for bb in range(BPC):
        xt, mask_t, adj_c = xts[bb], masks[bb], adjs[bb]

        # embed: hT[h, n] = W_embed.T @ xT
        h = h_pool.tile([HID, N], FP16, tag="h")
        for t in range(NT):
            ps = psA.tile([HID, 512], FP32, tag="psA")
            nc.tensor.matmul(ps[:], we_t[:], xt[:, t * 512:(t + 1) * 512],
                             start=True, stop=True)
            nc.scalar.copy(h[:, t * 512:(t + 1) * 512], ps[:])

        for i in range(NUM_LAYERS):
            # msg[n, k] = h @ Wl[i], natural layout chunks of 128 nodes
            msg_t = msg_pool.tile([128, NC8, HID], FP16, tag="msg")
            for c in range(NC8):
                pm = psM.tile([128, HID], FP32, tag="psM")
                nc.tensor.matmul(pm[:], h[:, c * 128:(c + 1) * 128], wl_t[i][:],
                                 start=True, stop=True)
                nc.vector.tensor_add(msg_t[:, c, :], pm[:], bl_t[i][:])
            # aggT[k, n] = msg.T @ adjT + bl[i] (x) rowsum ; hT = relu(aggT)
            h2 = h_pool.tile([HID, N], FP16, tag="h")
            for t in range(NT):
                ps = psA.tile([HID, 512], FP32, tag="psA")
                for c in range(NC8):
                    nc.tensor.matmul(ps[:], msg_t[:, c, :],
                                     adj_c[c][:, t * 512:(t + 1) * 512],
                                     start=(c == 0), stop=(c == NC8 - 1))
                nc.scalar.activation(h2[:, t * 512:(t + 1) * 512], ps[:], RELU,
                                     scale=S[i] / S[i + 1])
            h = h2

        # projection back to natural layout + bias + mask; batch the whole
        # graph's output into one SWDGE store (gpsimd DMAs take engine-level
        # waits, so producer deps don't overflow the HWDGE wait slot).
        o_big = o_pool.tile([128, NC8, OUT], FP32, tag="o")
        for c in range(NC8):
            po = psO.tile([128, OUT], FP32, tag="psO")
            nc.tensor.matmul(po[:], h[:, c * 128:(c + 1) * 128], wp_t[:],
                             start=True, stop=False)
            nc.tensor.matmul(po[:], ones_t[:], bp_t[:], start=False, stop=True)
            nc.scalar.activation(o_big[:, c, :], po[:], COPY,
                                 scale=mask_t[:, c:c + 1])
        nc.sync.dma_start(out[bb].rearrange("(c p) j -> p c j", p=128),
                          o_big[:])


def build_nc():
    # Bacc (not raw Bass): its compile() runs generate_event_semaphores,
    # which splits multi-sem waits down to the 1-wait-per-instruction
    # hardware limit walrus enforces.
    nc = bacc.Bacc("TRN2", debug=False, num_devices=N_CORES, num_swdge_queues=2)
    adjT = nc.dram_tensor("adjT", [BPC, N, N], FP16, kind="ExternalInput").ap()
    xT = nc.dram_tensor("xT", [BPC, IN_DIM, N], FP16, kind="ExternalInput").ap()
    maskT = nc.dram_tensor("maskT", [BPC, 128, NC8], FP32, kind="ExternalInput").ap()
    w_embed = nc.dram_tensor("w_embed", [IN_DIM, HID], FP16, kind="ExternalInput").ap()
    wl = nc.dram_tensor("wl", [NUM_LAYERS, HID, HID], FP16, kind="ExternalInput").ap()
    blT = nc.dram_tensor("blT", [NUM_LAYERS, 1, HID], FP32, kind="ExternalInput").ap()
    w_proj = nc.dram_tensor("w_proj", [HID, OUT], FP16, kind="ExternalInput").ap()
    b_proj = nc.dram_tensor("b_proj", [1, OUT], FP16, kind="ExternalInput").ap()
    out = nc.dram_tensor("out", [BPC, N, OUT], FP32, kind="ExternalOutput").ap()

    with tile.TileContext(nc) as tc, ExitStack() as ctx:
        _kernel_body(ctx, tc, out, adjT, xT, maskT,
                     w_embed, wl, blT, w_proj, b_proj)
    nc.compile()
    return nc


def make_in_maps(node_features, adjacency_matrix, node_mask, W_embed, Wl, bl,
                 W_proj, b_proj):
    x = np.asarray(node_features, dtype=np.float32)
    adj = np.asarray(adjacency_matrix, dtype=np.float32)
    mask = np.asarray(node_mask, dtype=np.float32)
    bl_scaled = np.asarray(bl, dtype=np.float64) / np.array(S[:NUM_LAYERS])[:, None]
    shared = {
        "w_embed": np.asarray(W_embed, dtype=np.float16),
        "wl": np.asarray(Wl, dtype=np.float16),
        "blT": bl_scaled.astype(np.float32).reshape(NUM_LAYERS, 1, HID),
        "w_proj": np.asarray(W_proj, dtype=np.float16),
        "b_proj": (np.asarray(b_proj, np.float64) / (ONES_VAL * S[NUM_LAYERS]))
        .astype(np.float16).reshape(1, OUT),
    }
    in_maps = []
    for c in range(N_CORES):
        sl = slice(c * BPC, (c + 1) * BPC)
        in_maps.append({
            "adjT": np.ascontiguousarray(
                adj[sl].transpose(0, 2, 1)).astype(np.float16),
            "xT": np.ascontiguousarray(x[sl].transpose(0, 2, 1)).astype(np.float16),
            "maskT": np.ascontiguousarray(
                mask[sl].reshape(BPC, NC8, 128).transpose(0, 2, 1))
            * np.float32(S[NUM_LAYERS]),
            **shared,
        })
    return in_maps


_NC_CACHE = None


def get_nc():
    global _NC_CACHE
    if _NC_CACHE is None:
        _NC_CACHE = build_nc()
    return _NC_CACHE


def kernel(**inputs):
    nc = get_nc()
    in_maps = make_in_maps(**inputs)
    res = run_bass_kernel_spmd(nc, in_maps, list(range(N_CORES)))
    outs = [np.asarray(res.results[c]["out"], dtype=np.float32)
            for c in range(N_CORES)]
    return np.concatenate(outs, axis=0)


if __name__ == "__main__":
    rng = np.random.default_rng(0)
    ins = {
        "node_features": rng.standard_normal((B, N, IN_DIM), dtype=np.float32),
        "adjacency_matrix": rng.random((B, N, N), dtype=np.float32),
        "node_mask": np.ones((B, N, 1), np.float32),
        "W_embed": rng.standard_normal((IN_DIM, HID), dtype=np.float32) * 0.1,
        "Wl": rng.standard_normal((NUM_LAYERS, HID, HID), dtype=np.float32) * 0.08,
        "bl": rng.standard_normal((NUM_LAYERS, HID), dtype=np.float32) * 0.08,
        "W_proj": rng.standard_normal((HID, 2 * 32), dtype=np.float32) * 0.08,
        "b_proj": rng.standard_normal((2 * 32,), dtype=np.float32) * 0.08,
    }
    out = kernel(**ins)
    print("out", out.shape, out.dtype, float(np.abs(out).mean()))
